# revision 1
# baseline (speedup 1.0000x reference)
"""Trainium2 Bass kernel for nn_AttentionHead (B=2, S=2048, D=768, H=12).

Sharding: 8 cores = 2 batches x 4 head-groups (3 heads each).
Per core: QKV projection for its heads (transposed layout), causal
attention with softmax over the QUERY axis (reference peculiarity:
softmax dim=-2, scaled by sqrt(d_model)), AllGather of per-head outputs
within each batch's 4-core group, then a column-slice of the output
projection.  Host only slices / transposes / concatenates.

Key layout choices:
  - Scores are built transposed: S_T[k, q] so the softmax axis (q) is
    the SBUF free axis; ScalarE exp computes the row sums for free via
    accum_out.  The per-k normalizer is folded into V ("V'") so the
    attn @ v matmul consumes raw exp scores.
  - Causal structure (checked on host) lets us skip ~40% of the score
    blocks; the 128-wide diagonal triangle is masked by a DVE add of a
    -1e30 constant tile into the PSUM scores before exp.
  - Matmul operands are bf16 (PSUM accumulation in f32); per-512-block
    AllGathers fire as the AV accumulator columns finalize so the
    collectives hide under compute.
"""

import math

import numpy as np

B, S, D, H, DH = 2, 2048, 768, 12, 64
NCORES = 8
GROUPS = 4  # head-groups per batch
HPG = 3  # heads per group
EPG = HPG * DH  # 192
SCALE = 1.0 / math.sqrt(D)
NEG = -1.0e30

_cache = {}


NMAX = 1024  # bf16 moving-operand max per matmul


def _build(causal: bool):
    import contextlib

    import concourse.bacc as bacc
    import concourse.mybir as mybir
    from concourse import tile

    f32 = mybir.dt.float32
    bf16 = mybir.dt.bfloat16
    EXP = mybir.ActivationFunctionType.Exp

    nc = bacc.Bacc("TRN2", target_bir_lowering=False, debug=False, num_devices=NCORES)

    xT = nc.dram_tensor("xT", [D, S], f32, kind="ExternalInput")
    wqk = nc.dram_tensor("wqk", [D, 384], f32, kind="ExternalInput")
    wv = nc.dram_tensor("wv", [D, EPG], f32, kind="ExternalInput")
    bqkc = nc.dram_tensor("bqkc", [384, 1], f32, kind="ExternalInput")
    bv = nc.dram_tensor("bv", [1, EPG], f32, kind="ExternalInput")
    wout = nc.dram_tensor("wout", [D, EPG], f32, kind="ExternalInput")
    boutc = nc.dram_tensor("boutc", [EPG, 1], f32, kind="ExternalInput")
    tri = nc.dram_tensor("tri", [128, 128], f32, kind="ExternalInput")
    ident = nc.dram_tensor("ident", [128, 128], f32, kind="ExternalInput")
    out = nc.dram_tensor("out", [EPG, S], f32, kind="ExternalOutput")

    ag_inA = [nc.dram_tensor(f"ag_inA{f}", [128, 512], bf16) for f in range(4)]
    ag_outA = [nc.dram_tensor(f"ag_outA{f}", [512, 512], bf16) for f in range(4)]
    ag_inB = [nc.dram_tensor(f"ag_inB{f}", [64, 1024], bf16) for f in range(2)]
    ag_outB = [nc.dram_tensor(f"ag_outB{f}", [256, 1024], bf16) for f in range(2)]

    groups = [[0, 1, 2, 3], [4, 5, 6, 7]]

    with tile.TileContext(nc) as tc:
        with contextlib.ExitStack() as ctx:
            const_p = ctx.enter_context(tc.tile_pool(name="const", bufs=1))
            w_p = ctx.enter_context(tc.tile_pool(name="w", bufs=6))
            qk_p = ctx.enter_context(tc.tile_pool(name="qk", bufs=1))
            v_p = ctx.enter_context(tc.tile_pool(name="v", bufs=1))
            e_p = ctx.enter_context(tc.tile_pool(name="e", bufs=10))
            st_p = ctx.enter_context(tc.tile_pool(name="stat", bufs=24))
            vp_p = ctx.enter_context(tc.tile_pool(name="vp", bufs=6))
            atn_p = ctx.enter_context(tc.tile_pool(name="atn", bufs=1))
            ag_p = ctx.enter_context(tc.tile_pool(name="ag", bufs=1))
            psS = ctx.enter_context(tc.tile_pool(name="psS", bufs=2, space="PSUM"))

            # ---- constants ----
            ones_f = const_p.tile([1, 512], f32)
            nc.vector.memset(ones_f[:], 1.0)
            ones = const_p.tile([1, 512], bf16)
            nc.vector.tensor_copy(ones[:], ones_f[:])
            tri_f = const_p.tile([128, 128], f32)
            nc.sync.dma_start(tri_f[:], tri[:, :])
            tri_b = const_p.tile([128, 128], bf16)
            nc.vector.tensor_copy(tri_b[:], tri_f[:])
            id_f = const_p.tile([128, 128], f32)
            nc.sync.dma_start(id_f[:], ident[:, :])
            id_b = const_p.tile([128, 128], bf16)
            nc.vector.tensor_copy(id_b[:], id_f[:])

            bqk_c = const_p.tile([128, 3], f32)
            nc.sync.dma_start(bqk_c[:], bqkc[:, :].rearrange("(c p) o -> p (c o)", p=128))
            bout_c = const_p.tile([128, 2], f32)
            nc.sync.dma_start(
                bout_c[0:64, 1:2], boutc[128:EPG, :]
            )
            nc.sync.dma_start(bout_c[:, 0:1], boutc[0:128, :])
            bv_f = const_p.tile([1, EPG], f32)
            nc.sync.dma_start(bv_f[:], bv[:, :])
            bv_t = const_p.tile([1, EPG], bf16)
            nc.vector.tensor_copy(bv_t[:], bv_f[:])

            # ---- PE warm-up: dummy matmuls while the x DMAs land ----
            warm_in = const_p.tile([128, 512], bf16)
            nc.vector.memset(warm_in[:], 0.0)
            for wi in range(16):
                wps = psS.tile([128, NMAX], f32, tag="strip")
                nc.tensor.matmul(
                    wps[:, 0:512], id_b[:], warm_in[:],
                    start=True, stop=True, skip_group_check=True,
                )

            # ---- xT first (DMA queue priority), then weights ----
            xt_ctx = tc.tile_pool(name="xt", bufs=1)
            xt_p = xt_ctx.__enter__()
            xs_ctx = tc.tile_pool(name="xs", bufs=6)
            xs_p = xs_ctx.__enter__()
            xt_t = []
            for dt_i in range(6):
                xs = xs_p.tile([128, S], f32, tag="xstg")
                nc.sync.dma_start(xs[:], xT[dt_i * 128 : (dt_i + 1) * 128, :])
                xt_tile = xt_p.tile([128, S], bf16, tag=f"xt{dt_i}")
                nc.vector.tensor_copy(xt_tile[:], xs[:])
                xt_t.append(xt_tile)
            xs_ctx.__exit__(None, None, None)

            wqk_t, wv_t, wout_t = [], [], []
            for dt_i in range(6):
                wf = w_p.tile([128, 384], f32, tag="wstg")
                nc.sync.dma_start(wf[:], wqk[dt_i * 128 : (dt_i + 1) * 128, :])
                wt = w_p.tile([128, 384], bf16, tag="wqk")
                nc.vector.tensor_copy(wt[:], wf[:])
                wqk_t.append(wt)
                vf = w_p.tile([128, EPG], f32, tag="wvstg")
                nc.sync.dma_start(vf[:], wv[dt_i * 128 : (dt_i + 1) * 128, :])
                vt = w_p.tile([128, EPG], bf16, tag="wv")
                nc.vector.tensor_copy(vt[:], vf[:])
                wv_t.append(vt)
                wos = w_p.tile([128, EPG], f32, tag="wostg")
                nc.sync.dma_start(wos[:], wout[dt_i * 128 : (dt_i + 1) * 128, :])
                wo = w_p.tile([128, EPG], bf16, tag="wout")
                nc.vector.tensor_copy(wo[:], wos[:])
                wout_t.append(wo)

            # qkv chunk: 2 s-chunks of 1024; bias folded into the copy
            def qkv_chunk(dst, c, sc):
                pt = psS.tile([128, NMAX], f32, tag="strip")
                for off in range(0, NMAX, 512):
                    for dt_i in range(6):
                        nc.tensor.matmul(
                            pt[:, off : off + 512],
                            wqk_t[dt_i][:, c * 128 : (c + 1) * 128],
                            xt_t[dt_i][:, sc * NMAX + off : sc * NMAX + off + 512],
                            start=(dt_i == 0), stop=(dt_i == 5),
                        )
                nc.vector.tensor_scalar_add(
                    dst[:, sc * NMAX : (sc + 1) * NMAX], pt[:], bqk_c[:, c : c + 1]
                )

            def v_tile(vnat, st_i):
                ptf = psS.tile([128, NMAX], f32, tag="strip")
                p = ptf[:, 0:EPG]
                nc.tensor.matmul(p, ones[:, 0:128], bv_t[:], start=True, stop=False)
                for dt_i in range(6):
                    nc.tensor.matmul(
                        p,
                        xt_t[dt_i][:, st_i * 128 : (st_i + 1) * 128],
                        wv_t[dt_i][:],
                        start=False, stop=(dt_i == 5),
                    )
                nc.vector.tensor_copy(vnat[:, st_i * EPG : (st_i + 1) * EPG], p)

            # ---- upfront: k01 + q01 ----
            k01 = qk_p.tile([128, S], bf16, tag="k01")
            q01 = qk_p.tile([128, S], bf16, tag="q01")
            qk2 = qk_p.tile([128, S], bf16, tag="qk2")
            k2 = qk_p.tile([64, S], bf16, tag="k2")
            q2d = qk_p.tile([128, S], bf16, tag="q2d")
            vnat = v_p.tile([128, 16 * EPG], bf16)
            for sc in range(2):
                qkv_chunk(qk2, 2, sc)
            nc.gpsimd.dma_start(k2[:], qk2[64:128, :])
            nc.gpsimd.dma_start(q2d[64:128, :], qk2[0:64, :])

            def halves_of(ki):
                q0 = 128 * ki if causal else 0
                L = S - q0
                hs = [(q0, min(L, NMAX))]
                if L > NMAX:
                    hs.append((q0 + NMAX, L - NMAX))
                return hs

            def strip_pair(ki, srcs, hv, h0, hl):
                """Emit the two row-group S matmuls adjacently, masks, exps.
                srcs: [(kT, kbase, qT, qbase), ...] for row groups lo/hi."""
                out_tiles = []
                s_list = []
                for (kT, kb, qT, qb) in srcs:
                    s_ps = psS.tile([128, NMAX], f32, tag="strip")
                    s_list.append(s_ps)
                off = 0
                while off < hl:
                    n = min(512, hl - off)
                    for s_ps, (kT, kb, qT, qb) in zip(s_list, srcs):
                        nc.tensor.matmul(
                            s_ps[:, off : off + n],
                            kT[kb : kb + 64, ki * 128 : (ki + 1) * 128],
                            qT[qb : qb + 64, h0 + off : h0 + off + n],
                            start=True,
                            stop=True,
                            skip_group_check=True,
                        )
                    off += n
                for s_ps in s_list:
                    if causal and hv == 0:
                        nc.vector.tensor_add(
                            s_ps[:, 0:128], s_ps[:, 0:128], tri_f[:]
                        )
                for s_ps in s_list:
                    et = e_p.tile([128, NMAX], bf16, tag="e")
                    acc = st_p.tile([128, 1], f32, tag="acc")
                    nc.scalar.activation(
                        et[:, 0:hl], s_ps[:, 0:hl], EXP,
                        scale=SCALE, accum_out=acc[:],
                    )
                    out_tiles.append((et, acc))
                return out_tiles

            def make_vpt(accs, ki, head):
                rcp = st_p.tile([128, 1], f32, tag="rcp")
                if len(accs) == 2:
                    ssum = st_p.tile([128, 1], f32, tag="ssum")
                    nc.vector.tensor_add(ssum[:], accs[0][:], accs[1][:])
                    nc.vector.reciprocal(rcp[:], ssum[:])
                else:
                    nc.vector.reciprocal(rcp[:], accs[0][:])
                vpt = vp_p.tile([128, 64], bf16, tag="vp")
                nc.vector.tensor_scalar_mul(
                    vpt[:],
                    vnat[:, ki * EPG + head * 64 : ki * EPG + (head + 1) * 64],
                    rcp[:],
                )
                return vpt

            # ---- wave A: heads 0+1 row/col paired, jobs interleaved ----
            def waveA(av_ps, atn, ag_dst, interleave):
                def flushA(f):
                    cols = slice(512 * f, 512 * (f + 1))
                    nc.vector.tensor_copy(atn[:, cols], av_ps[:, cols])
                    nc.sync.dma_start(ag_inA[f][:, :], atn[:, cols])
                    nc.gpsimd.collective_compute(
                        "AllGather",
                        mybir.AluOpType.bypass,
                        replica_groups=groups,
                        ins=[ag_inA[f].ap().opt()],
                        outs=[ag_outA[f].ap().opt()],
                    )
                    for dt_i in range(4):
                        nc.sync.dma_start(
                            agA_t[dt_i][:, cols],
                            ag_outA[f][dt_i * 128 : (dt_i + 1) * 128, :],
                        )

                for ki in range(16):
                    hs = halves_of(ki)
                    ets = {0: [], 1: []}
                    accs = {0: [], 1: []}
                    for hv, (h0, hl) in enumerate(hs):
                        res = strip_pair(
                            ki,
                            [(k01, 0, q01, 0), (k01, 64, q01, 64)],
                            hv, h0, hl,
                        )
                        for hi, (et, acc) in enumerate(res):
                            ets[hi].append((et, h0, hl))
                            accs[hi].append(acc)
                    for job in interleave.get(ki, []):
                        job()
                    vpts = [make_vpt(accs[hi], ki, hi) for hi in range(2)]
                    for hv, (h0, hl) in enumerate(hs):
                        off = 0
                        while off < hl:
                            n = min(512, hl - off)
                            for hi in range(2):
                                p_lo = 0 if hi == 0 else 64
                                et = ets[hi][hv][0]
                                nc.tensor.matmul(
                                    av_ps[p_lo : p_lo + 64, h0 + off : h0 + off + n],
                                    vpts[hi][:],
                                    et[:, off : off + n],
                                    start=(ki == 0),
                                    stop=(ki == 15),
                                    skip_group_check=True,
                                )
                            off += n
                    if causal and ki % 4 == 3:
                        flushA(ki // 4)
                if not causal:
                    for f in range(4):
                        flushA(f)

            # ---- wave B: head 2, ki pairs in row groups; AV serial ----
            def waveB(av_ps, atn, ag_dst, interleave):
                def flushB(f):
                    cols = slice(512 * f, 512 * (f + 1))
                    nc.vector.tensor_copy(atn[:, cols], av_ps[0:64, cols])
                    if f % 2 == 0:
                        return
                    g = f // 2
                    gcols = slice(1024 * g, 1024 * (g + 1))
                    nc.sync.dma_start(ag_inB[g][:, :], atn[:, gcols])
                    nc.gpsimd.collective_compute(
                        "AllGather",
                        mybir.AluOpType.bypass,
                        replica_groups=groups,
                        ins=[ag_inB[g].ap().opt()],
                        outs=[ag_outB[g].ap().opt()],
                    )
                    for dt_i in range(2):
                        nc.sync.dma_start(
                            agB_t[dt_i][:, gcols],
                            ag_outB[g][dt_i * 128 : (dt_i + 1) * 128, :],
                        )

                for t in range(8):
                    for job in interleave.get(t, []):
                        job()
                    kis = (2 * t, 2 * t + 1)
                    srcs = {
                        kis[0]: (k2, 0, qk2, 0),
                        kis[1]: (qk2, 64, q2d, 64),
                    }
                    ets = {ki: [] for ki in kis}
                    accs = {ki: [] for ki in kis}
                    maxhv = max(len(halves_of(ki)) for ki in kis)
                    for hv in range(maxhv):
                        batch = []
                        for ki in kis:
                            hs = halves_of(ki)
                            if hv < len(hs):
                                batch.append((ki, hs[hv]))
                        s_list = []
                        for ki, (h0, hl) in batch:
                            s_ps = psS.tile([128, NMAX], f32, tag="strip")
                            s_list.append(s_ps)
                        maxhl = max(hl for _, (_, hl) in batch)
                        off = 0
                        while off < maxhl:
                            for s_ps, (ki, (h0, hl)) in zip(s_list, batch):
                                if off >= hl:
                                    continue
                                n = min(512, hl - off)
                                kT, kb, qT, qb = srcs[ki]
                                nc.tensor.matmul(
                                    s_ps[:, off : off + n],
                                    kT[kb : kb + 64, ki * 128 : (ki + 1) * 128],
                                    qT[qb : qb + 64, h0 + off : h0 + off + n],
                                    start=True,
                                    stop=True,
                                    skip_group_check=True,
                                )
                            off += 512
                        for s_ps, (ki, (h0, hl)) in zip(s_list, batch):
                            if causal and hv == 0:
                                nc.vector.tensor_add(
                                    s_ps[:, 0:128], s_ps[:, 0:128], tri_f[:]
                                )
                        for s_ps, (ki, (h0, hl)) in zip(s_list, batch):
                            et = e_p.tile([128, NMAX], bf16, tag="e")
                            acc = st_p.tile([128, 1], f32, tag="acc")
                            nc.scalar.activation(
                                et[:, 0:hl], s_ps[:, 0:hl], EXP,
                                scale=SCALE, accum_out=acc[:],
                            )
                            ets[ki].append((et, h0, hl))
                            accs[ki].append(acc)
                    for ki in kis:
                        vpt = make_vpt(accs[ki], ki, 2)
                        for et, h0, hl in ets[ki]:
                            off = 0
                            while off < hl:
                                n = min(512, hl - off)
                                nc.tensor.matmul(
                                    av_ps[0:64, h0 + off : h0 + off + n],
                                    vpt[:],
                                    et[:, off : off + n],
                                    start=(ki == 0),
                                    stop=(ki == 15),
                                    skip_group_check=True,
                                )
                                off += n
                    if causal and t % 2 == 1:
                        flushB(t // 2)
                if not causal:
                    for f in range(4):
                        flushB(f)

            jobsB = {}
            for t in range(8):
                jobsB.setdefault(t, []).append(
                    lambda t=t: v_tile(vnat, 2 * t)
                )
                jobsB.setdefault(t, []).append(
                    lambda t=t: v_tile(vnat, 2 * t + 1)
                )
            jobsB.setdefault(0, []).append(lambda: qkv_chunk(q01, 1, 0))
            jobsB.setdefault(1, []).append(lambda: qkv_chunk(q01, 1, 1))
            jobsB.setdefault(2, []).append(lambda: qkv_chunk(k01, 0, 0))
            jobsB.setdefault(3, []).append(lambda: qkv_chunk(k01, 0, 1))

            agA_t = []
            for i in range(4):
                ag_tile = ag_p.tile([128, S], bf16, tag=f"agA{i}")
                agA_t.append(ag_tile)
            agB_t = []
            for i in range(2):
                ag_tile = ag_p.tile([128, S], bf16, tag=f"agB{i}")
                agB_t.append(ag_tile)

            psB_ctx = tc.tile_pool(name="psB", bufs=1, space="PSUM")
            psB = psB_ctx.__enter__()
            av2 = psB.tile([64, S], f32, tag="av2")
            atn2 = atn_p.tile([64, S], bf16, tag="atn2")
            waveB(av2, atn2, ag_inB, jobsB)
            psB_ctx.__exit__(None, None, None)
            xt_ctx.__exit__(None, None, None)

            psA_ctx = tc.tile_pool(name="psA", bufs=1, space="PSUM")
            psA = psA_ctx.__enter__()
            avA = psA.tile([128, S], f32, tag="avA")
            atnA = atn_p.tile([128, S], bf16, tag="atnA")
            waveA(avA, atnA, ag_inA, {})
            psA_ctx.__exit__(None, None, None)

            # ---- output projection: two-pass with SBUF partials ----
            o_p = ctx.enter_context(tc.tile_pool(name="o", bufs=1))
            ps2 = ctx.enter_context(tc.tile_pool(name="ps2", bufs=2, space="PSUM"))
            oT0 = o_p.tile([128, S], f32, tag="o0")
            oT1 = o_p.tile([64, S], f32, tag="o1")
            for sc in range(2):
                for mc, (m0, mw, dst) in enumerate([(0, 128, oT0), (128, 64, oT1)]):
                    pt = ps2.tile([128, NMAX], f32, tag="po")
                    for off in range(0, NMAX, 512):
                        for dt_i in range(4):
                            nc.tensor.matmul(
                                pt[0:mw, off : off + 512],
                                wout_t[dt_i][:, m0 : m0 + mw],
                                agA_t[dt_i][:, sc * NMAX + off : sc * NMAX + off + 512],
                                start=(dt_i == 0), stop=(dt_i == 3),
                            )
                    nc.vector.tensor_scalar_add(
                        dst[:, sc * NMAX : (sc + 1) * NMAX],
                        pt[0:mw, :],
                        bout_c[0:mw, mc : mc + 1],
                    )
                    pt2 = ps2.tile([128, NMAX], f32, tag="po")
                    for off in range(0, NMAX, 512):
                        for di, dt_i in enumerate((4, 5)):
                            nc.tensor.matmul(
                                pt2[0:mw, off : off + 512],
                                wout_t[dt_i][:, m0 : m0 + mw],
                                agB_t[dt_i - 4][:, sc * NMAX + off : sc * NMAX + off + 512],
                                start=(di == 0), stop=(di == 1),
                            )
                    nc.vector.tensor_add(
                        dst[:, sc * NMAX : (sc + 1) * NMAX],
                        dst[:, sc * NMAX : (sc + 1) * NMAX],
                        pt2[0:mw, :],
                    )
                    nc.sync.dma_start(
                        out[m0 : m0 + mw, sc * NMAX : (sc + 1) * NMAX],
                        dst[:, sc * NMAX : (sc + 1) * NMAX],
                    )
    nc.compile()
    return nc


def _shards(x, mask, W_in, b_in, W_out, b_out):
    """Build per-core input maps (host-side sharding / layout prep)."""
    tri_np = np.where(
        np.arange(128)[None, :] < np.arange(128)[:, None], np.float32(NEG), 0.0
    ).astype(np.float32)
    # split-AllGather row order: rank pairs (h=3r,3r+1) then solos (h=3r+2)
    head_order = [0, 1, 3, 4, 6, 7, 9, 10, 2, 5, 8, 11]
    row_perm = np.concatenate([np.arange(h * 64, (h + 1) * 64) for h in head_order])
    in_maps = []
    for c in range(NCORES):
        b = c // GROUPS
        g = c % GROUPS
        hs = [3 * g, 3 * g + 1, 3 * g + 2]
        qc = [W_in[:, 64 * h : 64 * (h + 1)] for h in hs]
        kc = [W_in[:, D + 64 * h : D + 64 * (h + 1)] for h in hs]
        vc = W_in[:, 2 * D + 64 * hs[0] : 2 * D + 64 * (hs[2] + 1)]
        qb = [b_in[64 * h : 64 * (h + 1)] for h in hs]
        kb = [b_in[D + 64 * h : D + 64 * (h + 1)] for h in hs]
        vb = b_in[2 * D + 64 * hs[0] : 2 * D + 64 * (hs[2] + 1)]
        wqk = np.concatenate(
            [kc[0], kc[1], qc[0], qc[1], qc[2], kc[2]], axis=1
        ).astype(np.float32)
        bqk = np.concatenate([kb[0], kb[1], qb[0], qb[1], qb[2], kb[2]])
        in_maps.append(
            {
                "xT": np.ascontiguousarray(x[b].T, dtype=np.float32),
                "wqk": np.ascontiguousarray(wqk),
                "wv": np.ascontiguousarray(vc, dtype=np.float32),
                "bqkc": np.ascontiguousarray(bqk[:, None], dtype=np.float32),
                "bv": np.ascontiguousarray(vb[None, :], dtype=np.float32),
                "wout": np.ascontiguousarray(
                    W_out[row_perm, EPG * g : EPG * (g + 1)], dtype=np.float32
                ),
                "boutc": np.ascontiguousarray(
                    b_out[EPG * g : EPG * (g + 1), None], dtype=np.float32
                ),
                "tri": tri_np,
                "ident": np.eye(128, dtype=np.float32),
            }
        )
    return in_maps


def _numpy_ref(x, mask, W_in, b_in, W_out, b_out):
    qkv = x @ W_in + b_in
    q, k, v = np.split(qkv, 3, axis=2)
    q = q.reshape(B, S, H, DH).transpose(0, 2, 1, 3)
    k = k.reshape(B, S, H, DH).transpose(0, 2, 1, 3)
    v = v.reshape(B, S, H, DH).transpose(0, 2, 1, 3)
    attn = np.einsum("bhqd,bhkd->bhqk", q, k) / np.sqrt(np.float32(D))
    attn = np.where(mask == 0, -np.inf, attn)
    attn = attn - attn.max(axis=-2, keepdims=True)
    e = np.exp(attn)
    attn = e / e.sum(axis=-2, keepdims=True)
    out = np.einsum("bhqk,bhkd->bhqd", attn, v)
    out = out.transpose(0, 2, 1, 3).reshape(B, S, D)
    return (out @ W_out + b_out).astype(np.float32)


def _run(inputs, trace=False):
    from concourse.bass_utils import run_bass_kernel_spmd

    x = np.asarray(inputs["x"], dtype=np.float32)
    mask = np.asarray(inputs["mask"])
    W_in = np.asarray(inputs["W_in"], dtype=np.float32)
    b_in = np.asarray(inputs["b_in"], dtype=np.float32)
    W_out = np.asarray(inputs["W_out"], dtype=np.float32)
    b_out = np.asarray(inputs["b_out"], dtype=np.float32)

    m2 = np.asarray(mask).reshape(S, S)
    if np.array_equal(m2, np.tril(np.ones((S, S), m2.dtype))):
        causal = True
    elif np.array_equal(m2, np.ones((S, S), m2.dtype)):
        causal = False
    else:
        return _numpy_ref(x, mask, W_in, b_in, W_out, b_out), None

    key = ("nc", causal)
    if key not in _cache:
        _cache[key] = _build(causal)
    nc = _cache[key]

    in_maps = _shards(x, mask, W_in, b_in, W_out, b_out)
    res = run_bass_kernel_spmd(nc, in_maps, core_ids=list(range(NCORES)), trace=trace)

    full = np.empty((B, S, D), dtype=np.float32)
    for c in range(NCORES):
        b, g = c // GROUPS, c % GROUPS
        full[b, :, EPG * g : EPG * (g + 1)] = res.results[c]["out"].T
    return full, res


def kernel(**inputs) -> np.ndarray:
    out, _ = _run(inputs, trace=False)
    return out



# revision 9
# speedup vs baseline: 1.1295x; 1.1295x over previous
"""Trainium2 Bass kernel for nn_AttentionHead (B=2, S=2048, D=768, H=12).

Sharding: 8 cores = 2 batches x 4 head-groups (3 heads each).
Per core: QKV projection for its heads (transposed layout), causal
attention with softmax over the QUERY axis (reference peculiarity:
softmax dim=-2, scaled by sqrt(d_model)), AllGather of per-head outputs
within each batch's 4-core group, then a column-slice of the output
projection.  Host only slices / transposes / casts / concatenates.

v2 layout/schedule notes:
  - All large inputs are pre-cast to bf16 on the host and DMA'd directly
    into matmul-ready SBUF tiles (no on-chip casts; half the HBM bytes).
  - Weight DMAs are issued before / interleaved with x so the first QKV
    matmul is not queued behind the full activation transfer.
  - Scores are built transposed (S_T[k, q]) so the softmax axis (q) is
    the SBUF free axis; ScalarE exp computes per-k row sums via
    accum_out.  The per-k normalizer is folded into V ("V'") so the
    attn @ v matmul consumes raw exp scores.
  - The q-projection bias is dropped: a per-k-row constant shifts all
    logits in the softmax (query) axis equally and cancels exactly.
  - Wave B (head 2) runs first, with the q01/k01 QKV chunks and the 16
    v-tiles interleaved as PE filler; per-512-column flushes fire four
    small AllGathers early.  Wave A (heads 0+1 row-paired) runs second,
    and the output projection is interleaved into the wave as AllGather
    results land, instead of running serially at the end.
  - The causal-diagonal mask add and the PSUM->SBUF copies run on the
    (otherwise idle) Pool engine to keep DVE off the critical path.
"""

import math

import numpy as np

B, S, D, H, DH = 2, 2048, 768, 12, 64
NCORES = 8
GROUPS = 4  # head-groups per batch
HPG = 3  # heads per group
EPG = HPG * DH  # 192
SCALE = 1.0 / math.sqrt(D)
NEG = -1.0e30

_cache = {}


def _build():
    import contextlib

    import concourse.bacc as bacc
    import concourse.mybir as mybir
    from concourse import tile

    f32 = mybir.dt.float32
    bf16 = mybir.dt.bfloat16
    EXP = mybir.ActivationFunctionType.Exp

    nc = bacc.Bacc("TRN2", target_bir_lowering=False, debug=False, num_devices=NCORES)

    xT = nc.dram_tensor("xT", [D, S], bf16, kind="ExternalInput")
    wqk = nc.dram_tensor("wqk", [D, 384], bf16, kind="ExternalInput")
    wv = nc.dram_tensor("wv", [D, EPG], bf16, kind="ExternalInput")
    wout = nc.dram_tensor("wout", [D, EPG], bf16, kind="ExternalInput")
    bqkc = nc.dram_tensor("bqkc", [128, 2], f32, kind="ExternalInput")
    bvv = nc.dram_tensor("bvv", [1, EPG], bf16, kind="ExternalInput")
    boutc = nc.dram_tensor("boutc", [128, 2], f32, kind="ExternalInput")
    tri = nc.dram_tensor("tri", [128, 128], bf16, kind="ExternalInput")
    ident = nc.dram_tensor("ident", [128, 128], bf16, kind="ExternalInput")
    out = nc.dram_tensor("out", [EPG, S], f32, kind="ExternalOutput")

    ag_inA = [nc.dram_tensor(f"ag_inA{f}", [128, 512], bf16) for f in range(4)]
    ag_outA = [nc.dram_tensor(f"ag_outA{f}", [512, 512], bf16) for f in range(4)]
    ag_inB = [nc.dram_tensor(f"ag_inB{f}", [64, 512], bf16) for f in range(4)]
    ag_outB = [nc.dram_tensor(f"ag_outB{f}", [256, 512], bf16) for f in range(4)]

    groups = [[0, 1, 2, 3], [4, 5, 6, 7]]

    with tile.TileContext(nc) as tc:
        with contextlib.ExitStack() as ctx:
            const_p = ctx.enter_context(tc.tile_pool(name="const", bufs=1))
            w_p = ctx.enter_context(tc.tile_pool(name="w", bufs=1))
            xt_p = ctx.enter_context(tc.tile_pool(name="xt", bufs=1))
            qk_p = ctx.enter_context(tc.tile_pool(name="qk", bufs=1))
            v_p = ctx.enter_context(tc.tile_pool(name="v", bufs=1))
            e_p = ctx.enter_context(tc.tile_pool(name="e", bufs=10))
            st_p = ctx.enter_context(tc.tile_pool(name="stat", bufs=24))
            vp_p = ctx.enter_context(tc.tile_pool(name="vp", bufs=6))
            atn_p = ctx.enter_context(tc.tile_pool(name="atn", bufs=1))
            ag_p = ctx.enter_context(tc.tile_pool(name="ag", bufs=1))
            o_p = ctx.enter_context(tc.tile_pool(name="o", bufs=1))

            # ---- constants (tiny DMAs first) ----
            bqk_c = const_p.tile([128, 2], f32)
            nc.sync.dma_start(bqk_c[:], bqkc[:, :])
            bout_c = const_p.tile([128, 2], f32)
            nc.sync.dma_start(bout_c[:], boutc[:, :])
            tri_b = const_p.tile([128, 128], bf16)
            nc.sync.dma_start(tri_b[:], tri[:, :])
            id_b = const_p.tile([128, 128], bf16)
            nc.sync.dma_start(id_b[:], ident[:, :])
            bv_t = const_p.tile([1, EPG], bf16)
            nc.sync.dma_start(bv_t[:], bvv[:, :])
            ones = const_p.tile([1, 512], bf16)
            nc.vector.memset(ones[:], 1.0)
            warm = const_p.tile([128, 512], bf16)
            nc.vector.memset(warm[:], 0.0)

            # ---- input DMAs: wqk first, then x halves, then wv, wout ----
            wqk_t = []
            for dt_i in range(6):
                wt = w_p.tile([128, 384], bf16, tag=f"wqk{dt_i}")
                nc.sync.dma_start(wt[:], wqk[dt_i * 128 : (dt_i + 1) * 128, :])
                wqk_t.append(wt)
            xt_t = []
            for dt_i in range(6):
                xt = xt_p.tile([128, S], bf16, tag=f"xt{dt_i}")
                nc.sync.dma_start(
                    xt[:, 0:1024], xT[dt_i * 128 : (dt_i + 1) * 128, 0:1024]
                )
                xt_t.append(xt)
            for dt_i in range(6):
                nc.sync.dma_start(
                    xt_t[dt_i][:, 1024:2048],
                    xT[dt_i * 128 : (dt_i + 1) * 128, 1024:2048],
                )
            wv_t, wout_t = [], []
            for dt_i in range(6):
                vt = w_p.tile([128, EPG], bf16, tag=f"wv{dt_i}")
                nc.sync.dma_start(vt[:], wv[dt_i * 128 : (dt_i + 1) * 128, :])
                wv_t.append(vt)
            for dt_i in range(6):
                wo = w_p.tile([128, EPG], bf16, tag=f"wout{dt_i}")
                nc.sync.dma_start(wo[:], wout[dt_i * 128 : (dt_i + 1) * 128, :])
                wout_t.append(wo)

            # ---- persistent SBUF tiles ----
            k01 = qk_p.tile([128, S], bf16, tag="k01")
            q01 = qk_p.tile([128, S], bf16, tag="q01")
            qk2 = qk_p.tile([128, S], bf16, tag="qk2")
            k2 = qk_p.tile([64, S], bf16, tag="k2")
            q2d = qk_p.tile([128, S], bf16, tag="q2d")
            vnat = v_p.tile([128, 16 * EPG], bf16)
            atnA = atn_p.tile([128, S], bf16, tag="atnA")
            atnB = atn_p.tile([64, S], bf16, tag="atnB")
            agA_t = [ag_p.tile([128, S], bf16, tag=f"agA{i}", name=f"agA{i}") for i in range(4)]
            agB_t = [ag_p.tile([128, S], bf16, tag=f"agB{i}", name=f"agB{i}") for i in range(2)]
            oT0 = o_p.tile([128, S], f32, tag="o0")
            oT1 = o_p.tile([64, S], f32, tag="o1")

            # ---- phase-B PSUM pools ----
            arB_ctx = tc.tile_pool(name="arB", bufs=3, space="PSUM")
            arB = arB_ctx.__enter__()
            avB_ctx = tc.tile_pool(name="avB", bufs=1, space="PSUM")
            avB = avB_ctx.__enter__()
            av2t = [avB.tile([128, 512], f32, tag=f"av2_{i}", name=f"av2_{i}") for i in range(2)]

            arena = [arB]

            # ---- PE warm-up while first DMAs land ----
            for _ in range(6):
                wps = arena[0].tile([128, 1024], f32, tag="strip")
                nc.tensor.matmul(
                    wps[:, 0:512], warm[:, 0:128], warm[:],
                    start=True, stop=True, skip_group_check=True,
                )

            # qkv chunk: column group c (128 wide), seq chunk sc (1024 wide)
            def qkv_chunk(dst, c, sc, bias_col):
                pt = arena[0].tile([128, 1024], f32, tag="strip")
                for off in range(0, 1024, 512):
                    for dt_i in range(6):
                        nc.tensor.matmul(
                            pt[:, off : off + 512],
                            wqk_t[dt_i][:, c * 128 : (c + 1) * 128],
                            xt_t[dt_i][:, sc * 1024 + off : sc * 1024 + off + 512],
                            start=(dt_i == 0), stop=(dt_i == 5),
                        )
                if bias_col is None:
                    nc.vector.tensor_copy(dst[:, sc * 1024 : (sc + 1) * 1024], pt[:])
                else:
                    nc.vector.tensor_scalar_add(
                        dst[:, sc * 1024 : (sc + 1) * 1024],
                        pt[:],
                        bqk_c[:, bias_col : bias_col + 1],
                    )

            def v_tile(st_i):
                pt = arena[0].tile([128, 1024], f32, tag="strip")
                p = pt[:, 0:EPG]
                nc.tensor.matmul(p, ones[:, 0:128], bv_t[:], start=True, stop=False)
                for dt_i in range(6):
                    nc.tensor.matmul(
                        p,
                        xt_t[dt_i][:, st_i * 128 : (st_i + 1) * 128],
                        wv_t[dt_i][:],
                        start=False, stop=(dt_i == 5),
                    )
                nc.scalar.copy(vnat[:, st_i * EPG : (st_i + 1) * EPG], p)

            # score strip for block ki: S_T[k, q] over q in [128*ki, S),
            # in parts of <=1024 cols; exp with accumulated row sums.
            def strip_emit(ki, kT, kb, qT, qb):
                q0 = 128 * ki
                L = S - q0
                parts = []
                j = 0
                while j * 1024 < L:
                    plen = min(1024, L - j * 1024)
                    t = arena[0].tile([128, 1024], f32, tag="strip")
                    off = 0
                    while off < plen:
                        n = min(512, plen - off)
                        nc.tensor.matmul(
                            t[:, off : off + n],
                            kT[kb : kb + 64, ki * 128 : (ki + 1) * 128],
                            qT[qb : qb + 64, q0 + j * 1024 + off : q0 + j * 1024 + off + n],
                            start=True, stop=True, skip_group_check=True,
                        )
                        off += n
                    if j == 0:
                        # causal diagonal mask: accumulate triT (-1e30 above
                        # diagonal) onto the diag score block via the PE
                        nc.tensor.matmul(
                            t[:, 0:128], tri_b[:], id_b[:],
                            start=False, stop=True, skip_group_check=True,
                        )
                    et = e_p.tile([128, 1024], bf16, tag="e")
                    acc = st_p.tile([128, 1], f32, tag="acc")
                    nc.scalar.activation(
                        et[:, 0:plen], t[:, 0:plen], EXP,
                        scale=SCALE, accum_out=acc[:],
                    )
                    parts.append((et, q0 + j * 1024, plen, acc))
                    j += 1
                return parts

            def make_vpt(parts, ki, head):
                accs = [p[3] for p in parts]
                a = accs[0]
                for extra in accs[1:]:
                    s = st_p.tile([128, 1], f32, tag="ssum")
                    nc.vector.tensor_add(s[:], a[:], extra[:])
                    a = s
                rcp = st_p.tile([128, 1], f32, tag="rcp")
                nc.vector.reciprocal(rcp[:], a[:])
                vpt = vp_p.tile([128, 64], bf16, tag="vp")
                nc.vector.tensor_scalar_mul(
                    vpt[:],
                    vnat[:, ki * EPG + head * 64 : ki * EPG + (head + 1) * 64],
                    rcp[:],
                )
                return vpt

            # attn@v accumulate: av_slice(b) -> [64, 512] psum view of block b
            def av_emit(ki, vpt, parts, av_slice):
                for (et, qp0, plen, _acc) in parts:
                    q = qp0
                    while q < qp0 + plen:
                        b = q // 512
                        qe = min((b + 1) * 512, qp0 + plen)
                        dst = av_slice(b)
                        nc.tensor.matmul(
                            dst[:, q - 512 * b : qe - 512 * b],
                            vpt[:],
                            et[:, q - qp0 : qe - qp0],
                            start=(ki == 0), stop=(ki == 4 * b + 3),
                            skip_group_check=True,
                        )
                        q = qe

            def flushB(f):
                cols = slice(512 * f, 512 * (f + 1))
                src = av2t[f // 2][(f % 2) * 64 : (f % 2) * 64 + 64, :]
                nc.vector.tensor_copy(atnB[:, cols], src)
                nc.sync.dma_start(ag_inB[f][:, :], atnB[:, cols])
                nc.gpsimd.collective_compute(
                    "AllGather",
                    mybir.AluOpType.bypass,
                    replica_groups=groups,
                    ins=[ag_inB[f].ap().opt()],
                    outs=[ag_outB[f].ap().opt()],
                )
                for r in range(4):
                    nc.sync.dma_start(
                        agB_t[r // 2][64 * (r % 2) : 64 * (r % 2) + 64, cols],
                        ag_outB[f][r * 64 : (r + 1) * 64, :],
                    )

            def flushA(f):
                cols = slice(512 * f, 512 * (f + 1))
                nc.vector.tensor_copy(atnA[:, cols], av01[f][:, :])
                nc.sync.dma_start(ag_inA[f][:, :], atnA[:, cols])
                nc.gpsimd.collective_compute(
                    "AllGather",
                    mybir.AluOpType.bypass,
                    replica_groups=groups,
                    ins=[ag_inA[f].ap().opt()],
                    outs=[ag_outA[f].ap().opt()],
                )
                for r in range(4):
                    nc.sync.dma_start(
                        agA_t[r][:, cols], ag_outA[f][r * 128 : (r + 1) * 128, :]
                    )

            def outproj(f):
                cols = slice(512 * f, 512 * (f + 1))
                pt = arena[0].tile([128, 1024], f32, tag="strip")
                movs = [agA_t[0], agA_t[1], agA_t[2], agA_t[3], agB_t[0], agB_t[1]]
                for dt_i in range(6):
                    nc.tensor.matmul(
                        pt[:, 0:512],
                        wout_t[dt_i][:, 0:128],
                        movs[dt_i][:, cols],
                        start=(dt_i == 0), stop=(dt_i == 5),
                    )
                for dt_i in range(6):
                    nc.tensor.matmul(
                        pt[0:64, 512:1024],
                        wout_t[dt_i][:, 128:EPG],
                        movs[dt_i][:, cols],
                        start=(dt_i == 0), stop=(dt_i == 5),
                    )
                nc.vector.tensor_scalar_add(
                    oT0[:, cols], pt[:, 0:512], bout_c[:, 0:1]
                )
                nc.vector.tensor_scalar_add(
                    oT1[:, cols], pt[0:64, 512:1024], bout_c[0:64, 1:2]
                )
                nc.sync.dma_start(out[0:128, cols], oT0[:, cols])
                nc.sync.dma_start(out[128:EPG, cols], oT1[:, cols])

            # ---- QKV c=2 (qk2 = [q2 | k2]) then k2/q2d rearrangement ----
            qkv_chunk(qk2, 2, 0, 1)
            qkv_chunk(qk2, 2, 1, 1)
            nc.gpsimd.dma_start(k2[:], qk2[64:128, :])
            nc.gpsimd.dma_start(q2d[64:128, :], qk2[0:64, :])

            # ---- wave B: head 2, ki pairs per step, jobs interleaved ----
            jobsB = {
                0: [lambda: qkv_chunk(q01, 1, 0, None)],
                1: [lambda: qkv_chunk(q01, 1, 1, None)],
                2: [lambda: qkv_chunk(k01, 0, 0, 0)],
                3: [lambda: qkv_chunk(k01, 0, 1, 0)],
            }
            for t in range(8):
                jobsB.setdefault(t, []).append(lambda t=t: v_tile(2 * t))
                jobsB.setdefault(t, []).append(lambda t=t: v_tile(2 * t + 1))

            def av2_slice(b):
                return av2t[b // 2][(b % 2) * 64 : (b % 2) * 64 + 64, :]

            for t in range(8):
                kis = (2 * t, 2 * t + 1)
                srcs = {
                    kis[0]: (k2, 0, qk2, 0),
                    kis[1]: (qk2, 64, q2d, 64),
                }
                partsB = {}
                for ki in kis:
                    partsB[ki] = strip_emit(ki, *srcs[ki])
                for job in jobsB.get(t, []):
                    job()
                for ki in kis:
                    vpt = make_vpt(partsB[ki], ki, 2)
                    av_emit(ki, vpt, partsB[ki], av2_slice)
                if t % 2 == 1:
                    flushB(t // 2)

            avB_ctx.__exit__(None, None, None)
            arB_ctx.__exit__(None, None, None)

            # ---- phase-A PSUM pools ----
            arA_ctx = tc.tile_pool(name="arA", bufs=2, space="PSUM")
            arA = arA_ctx.__enter__()
            avA_ctx = tc.tile_pool(name="avA", bufs=1, space="PSUM")
            avA = avA_ctx.__enter__()
            av01 = [avA.tile([128, 512], f32, tag=f"av01_{i}", name=f"av01_{i}") for i in range(4)]
            arena[0] = arA

            def avA_slice(hi):
                def sl(b):
                    return av01[b][64 * hi : 64 * hi + 64, :]
                return sl

            # ---- wave A: heads 0+1 row-paired; out-proj interleaved ----
            srcsA = {0: (k01, 0, q01, 0), 1: (k01, 64, q01, 64)}
            for ki in range(16):
                partsA = {}
                for hi in (0, 1):
                    partsA[hi] = strip_emit(ki, *srcsA[hi])
                if ki == 8:
                    outproj(0)
                if ki == 12:
                    outproj(1)
                for hi in (0, 1):
                    vpt = make_vpt(partsA[hi], ki, hi)
                    av_emit(ki, vpt, partsA[hi], avA_slice(hi))
                if ki % 4 == 3:
                    flushA(ki // 4)
            outproj(2)
            outproj(3)

            avA_ctx.__exit__(None, None, None)
            arA_ctx.__exit__(None, None, None)

    nc.compile()
    return nc


def _shards(x, W_in, b_in, W_out, b_out):
    """Build per-core input maps (host-side sharding / layout / bf16 prep)."""
    import ml_dtypes

    bf16 = ml_dtypes.bfloat16
    tri_stat = np.where(
        np.arange(128)[:, None] < np.arange(128)[None, :], np.float32(NEG), 0.0
    ).astype(bf16)
    id_np = np.eye(128, dtype=np.float32).astype(bf16)
    # AllGather row order: rank pairs (h=3r, 3r+1) for wave A, then
    # solos (h=3r+2) for wave B.
    head_order = [0, 1, 3, 4, 6, 7, 9, 10, 2, 5, 8, 11]
    row_perm = np.concatenate([np.arange(h * 64, (h + 1) * 64) for h in head_order])
    zeros64 = np.zeros(64, dtype=np.float32)
    in_maps = []
    for c in range(NCORES):
        b = c // GROUPS
        g = c % GROUPS
        hs = [3 * g, 3 * g + 1, 3 * g + 2]
        qc = [W_in[:, 64 * h : 64 * (h + 1)] for h in hs]
        kc = [W_in[:, D + 64 * h : D + 64 * (h + 1)] for h in hs]
        vc = W_in[:, 2 * D + 64 * hs[0] : 2 * D + 64 * (hs[2] + 1)]
        kb = [b_in[D + 64 * h : D + 64 * (h + 1)] for h in hs]
        vb = b_in[2 * D + 64 * hs[0] : 2 * D + 64 * (hs[2] + 1)]
        # col order: [k0 k1 | q0 q1 | q2 k2]
        wqk = np.concatenate(
            [kc[0], kc[1], qc[0], qc[1], qc[2], kc[2]], axis=1
        )
        bqkc = np.stack(
            [np.concatenate([kb[0], kb[1]]), np.concatenate([zeros64, kb[2]])],
            axis=1,
        ).astype(np.float32)
        bo = b_out[EPG * g : EPG * (g + 1)]
        boutc = np.stack(
            [bo[0:128], np.concatenate([bo[128:EPG], zeros64])], axis=1
        ).astype(np.float32)
        in_maps.append(
            {
                "xT": np.ascontiguousarray(x[b].T).astype(bf16),
                "wqk": np.ascontiguousarray(wqk).astype(bf16),
                "wv": np.ascontiguousarray(vc).astype(bf16),
                "wout": np.ascontiguousarray(
                    W_out[row_perm, EPG * g : EPG * (g + 1)]
                ).astype(bf16),
                "bqkc": np.ascontiguousarray(bqkc),
                "bvv": np.ascontiguousarray(vb[None, :]).astype(bf16),
                "boutc": np.ascontiguousarray(boutc),
                "tri": tri_stat,
                "ident": id_np,
            }
        )
    return in_maps


def _numpy_ref(x, mask, W_in, b_in, W_out, b_out):
    qkv = x @ W_in + b_in
    q, k, v = np.split(qkv, 3, axis=2)
    q = q.reshape(B, S, H, DH).transpose(0, 2, 1, 3)
    k = k.reshape(B, S, H, DH).transpose(0, 2, 1, 3)
    v = v.reshape(B, S, H, DH).transpose(0, 2, 1, 3)
    attn = np.einsum("bhqd,bhkd->bhqk", q, k) / np.sqrt(np.float32(D))
    attn = np.where(mask == 0, -np.inf, attn)
    attn = attn - attn.max(axis=-2, keepdims=True)
    e = np.exp(attn)
    attn = e / e.sum(axis=-2, keepdims=True)
    out = np.einsum("bhqk,bhkd->bhqd", attn, v)
    out = out.transpose(0, 2, 1, 3).reshape(B, S, D)
    return (out @ W_out + b_out).astype(np.float32)


def _run(inputs, trace=False):
    from concourse.bass_utils import run_bass_kernel_spmd

    x = np.asarray(inputs["x"], dtype=np.float32)
    mask = np.asarray(inputs["mask"])
    W_in = np.asarray(inputs["W_in"], dtype=np.float32)
    b_in = np.asarray(inputs["b_in"], dtype=np.float32)
    W_out = np.asarray(inputs["W_out"], dtype=np.float32)
    b_out = np.asarray(inputs["b_out"], dtype=np.float32)

    m2 = np.asarray(mask).reshape(S, S)
    if not np.array_equal(m2, np.tril(np.ones((S, S), m2.dtype))):
        return _numpy_ref(x, mask, W_in, b_in, W_out, b_out), None

    if "nc" not in _cache:
        _cache["nc"] = _build()
    nc = _cache["nc"]

    in_maps = _shards(x, W_in, b_in, W_out, b_out)
    res = run_bass_kernel_spmd(nc, in_maps, core_ids=list(range(NCORES)), trace=trace)

    full = np.empty((B, S, D), dtype=np.float32)
    for c in range(NCORES):
        b, g = c // GROUPS, c % GROUPS
        full[b, :, EPG * g : EPG * (g + 1)] = res.results[c]["out"].T
    return full, res


def kernel(**inputs) -> np.ndarray:
    out, _ = _run(inputs, trace=False)
    return out
